# revision 21
# baseline (speedup 1.0000x reference)
"""CrossAttention Trainium2 kernel (8 NeuronCores, SPMD).

Sharding: 8 cores = batch(2) x query-block(4 x 1024). Each core computes a
[1024, 1024] slice of the output; no cross-core communication.

Reference math (per core, M=1024 query tokens, Skv=1024, D=1024, H=16, hd=64):
  q = hs @ Wq ; k = enc @ Wk ; v = enc @ Wv
  per-head LN(q), LN(k) over hd; scores = LN(q) @ LN(k)^T / sqrt(hd)
  out = softmax(scores) @ v ; return out @ Wo
Host folds the LN mean-centering into Wq/Wk (exact), pre-transposes
activations to feature-major, and casts matmul operands to bf16.

Schedule: K MMs -> V MMs (K LN-finalize interleaved) -> Q chunk-0 ->
Q chunk-1 MMs (chunk-0 finalize interleaved, 2-group lag so no rb matmul
waits on a just-issued DVE reciprocal) -> attention (exp on ScalarE is the
rate limiter at ~91% occupancy; scores are K=64 row-tiled so both heads
run concurrently; AV of quarter q-1 is emitted after the scores of q so
the in-order PE queue never stalls behind an exp; softmax sums ride the
augmented-ones column of V and are DMA-scattered to head rows; the
normalize pieces run inline one p-group behind) -> tail (O-projection).
A dummy exp pinned after the last Sqrt preloads the exp ACT table so the
table load cannot starve the PE into a HAM re-throttle at attention start.
"""

import numpy as np
import ml_dtypes
from contextlib import ExitStack

import concourse.bass as bass
import concourse.tile as tile
from concourse import bacc, mybir
from concourse.bass_utils import run_bass_kernel_spmd

BF = mybir.dt.bfloat16
F32 = mybir.dt.float32
F32R = mybir.dt.float32r

D = 1024      # model dim
H = 16        # heads
HD = 64       # head dim
M = 1024      # query tokens per core
SKV = 1024    # kv tokens (one batch)
B = 2
SQ = 4096
NCORES = 8
LN_EPS = 1e-5
CH = 512      # query-token chunk

_cache = {}


def _selector_constants():
    # sel16[d][p, j]: 1 if head j == 2d + p//64  (sum-over-head-partitions lhsT)
    sel16 = np.zeros((8, 128, H), np.float32)
    for d in range(8):
        for p in range(128):
            sel16[d, p, 2 * d + p // 64] = 1.0
    # selB[d][j, p]: 1 if head j == 2d + p//64  (broadcast-to-head-partitions lhsT)
    selB = np.transpose(sel16, (0, 2, 1)).copy()
    return sel16, selB


def _emit(ctx: ExitStack, tc, t, has_bias_q, has_bias_k):
    nc = tc.nc

    persist = ctx.enter_context(tc.tile_pool(name="persist", bufs=1))

    # ---- persistent SBUF tensors (stacked [128, 8, 1024] layout) ----
    hst = persist.tile([128, 8, M], BF, tag="hst")       # hs^T  (feature-major)
    enct = persist.tile([128, 8, SKV], BF, tag="enct")   # enc^T (feature-major)
    wq = persist.tile([128, 8, D], BF, tag="wq")
    wk = persist.tile([128, 8, D], BF, tag="wk")
    wv = persist.tile([128, 8, D], BF, tag="wv")
    wo = persist.tile([128, 8, D], BF, tag="wo")
    qtln = persist.tile([128, 8, M], BF, tag="qtln")     # LN(q)^T feature-major
    ktln = persist.tile([128, 8, SKV], BF, tag="ktln")   # LN(k)^T feature-major
    vaug = persist.tile([128, 8, H, HD + 1], BF, tag="vaug")  # [kv, h, V|1]
    aout = persist.tile([128, 8, M], BF, tag="aout")     # attn out^T feature-major
    gq_sb = persist.tile([128, 8], F32, tag="gq_sb")
    gk_sb = persist.tile([128, 8], F32, tag="gk_sb")
    sel16_sb = persist.tile([128, 8, H], F32R, tag="sel16_sb")
    selB_sb = persist.tile([16, 8, 128], F32R, tag="selB_sb")
    rinv_q = persist.tile([16, M], F32R, tag="rinv_q")    # 1/std per (head, tok)
    rinv_k = persist.tile([16, SKV], F32R, tag="rinv_k")
    sums_sb = persist.tile([16, M], F32, tag="sums_sb")   # softmax sums
    inv_s = persist.tile([16, M], F32R, tag="inv_s")      # 1/softmax-sum
    eps_sb = persist.tile([16, 1], F32, tag="eps_sb")
    nc.vector.memset(eps_sb[:, :], LN_EPS)
    nc.vector.memset(sums_sb[:, :], 1.0)  # recip_fast is undefined on junk
    nc.vector.memset(vaug[:, :, :, HD:HD + 1], 1.0)
    bq_sb = persist.tile([128, 8], F32, tag="bq_sb") if has_bias_q else None
    bk_sb = persist.tile([128, 8], F32, tag="bk_sb") if has_bias_k else None

    # ---- loads ----
    for k in range(8):
        nc.sync.dma_start(enct[:, k, :], t["encT"][k * 128:(k + 1) * 128, :])
        nc.sync.dma_start(wk[:, k, :], t["wk"][k * 128:(k + 1) * 128, :])
    for k in range(8):
        nc.sync.dma_start(wv[:, k, :], t["wv"][k * 128:(k + 1) * 128, :])
        nc.sync.dma_start(hst[:, k, :], t["hsT"][k * 128:(k + 1) * 128, :])
    for k in range(8):
        nc.sync.dma_start(wq[:, k, :], t["wq"][k * 128:(k + 1) * 128, :])
        nc.sync.dma_start(wo[:, k, :], t["wo"][k * 128:(k + 1) * 128, :])
    nc.sync.dma_start(gq_sb[:, :], t["gq"].rearrange("(d p) -> p d", p=128))
    nc.sync.dma_start(gk_sb[:, :], t["gk"].rearrange("(d p) -> p d", p=128))
    if has_bias_q:
        nc.sync.dma_start(bq_sb[:, :], t["bq"].rearrange("(d p) -> p d", p=128))
    if has_bias_k:
        nc.sync.dma_start(bk_sb[:, :], t["bk"].rearrange("(d p) -> p d", p=128))
    nc.sync.dma_start(sel16_sb[:, :, :], t["sel16"].rearrange("d p j -> p d j"))
    nc.sync.dma_start(selB_sb[:, :, :], t["selB"].rearrange("d j p -> j d p"))

    def recip_fast(out_ap, in_ap):
        from concourse.dve_ops import (
            RECIP_APPROX_FAST_CONSTS,
            RECIPROCAL_APPROX_FAST,
        )
        c = RECIP_APPROX_FAST_CONSTS
        nc.vector._custom_dve(
            RECIPROCAL_APPROX_FAST, out=out_ap, in0=in_ap,
            s0=c["s0"], s1=c["s1"], imm2=c["imm2"],
        )

    sq_pool = ctx.enter_context(tc.tile_pool(name="sq_pool", bufs=2))
    rstd_pool = ctx.enter_context(tc.tile_pool(name="rstd_pool", bufs=2))

    # ---------------- projection helpers ----------------
    def proj_mms(ps_proj, ps_ssq, w_sb, x_sb, ln_sb, cg, stage_act):
        """Emit projection matmuls + staging + per-head sum-of-squares for
        token chunks cg. Returns the ssq PSUM tiles (per chunk)."""
        ssqs = {}
        for c in cg:
            ssqs[c] = ps_ssq.tile([16, 512], F32, tag="ssq", name=f"ssq{c}")
        for d in range(8):
            accs = {c: ps_proj.tile([128, 512], F32, tag="acc",
                                    name=f"acc{c}") for c in cg}
            for k in range(8):
                for c in cg:
                    nc.tensor.matmul(
                        accs[c][:, :],
                        lhsT=w_sb[:, k, d * 128:(d + 1) * 128],
                        rhs=x_sb[:, k, c * 512:(c + 1) * 512],
                        start=(k == 0), stop=(k == 7),
                    )
            for c in cg:
                acc = accs[c]
                # stage raw projection (bf16); LN apply rescales in place
                if stage_act:
                    nc.scalar.copy(ln_sb[:, d, c * 512:(c + 1) * 512],
                                   acc[:, :])
                else:
                    nc.vector.tensor_copy(
                        ln_sb[:, d, c * 512:(c + 1) * 512], acc[:, :])
                sq = sq_pool.tile([128, 512], F32R)
                nc.vector.tensor_mul(sq[:, :],
                                     ln_sb[:, d, c * 512:(c + 1) * 512],
                                     ln_sb[:, d, c * 512:(c + 1) * 512])
                nc.tensor.matmul(
                    ssqs[c][:, :],
                    lhsT=sel16_sb[:, d, :],
                    rhs=sq[:, :],
                    start=(d == 0), stop=(d == 7),
                    skip_group_check=True,
                )
        return ssqs

    def proj_rstd(ssq, rinv_sb, c):
        # rstd = sqrt(ssq/64 + eps); rinv = 1/rstd
        rstd = rstd_pool.tile([16, 512], F32)
        nc.scalar.activation(
            rstd[:, :], ssq[:, :], mybir.ActivationFunctionType.Sqrt,
            bias=eps_sb[:, :], scale=1.0 / HD,
        )
        recip_fast(rinv_sb[:, c * 512:(c + 1) * 512], rstd[:, :])
        return rstd

    def proj_fin_d(ps_rb, rb_tag, ln_sb, g_sb, b_sb, rinv_sb, c, d):
        # apply: ln = raw * g * rinv (+ b) for feature block d of chunk c
        rb = ps_rb.tile([128, 512], F32, tag=rb_tag)
        nc.tensor.matmul(
            rb[:, :],
            lhsT=selB_sb[:, d, :],
            rhs=rinv_sb[:, c * 512:(c + 1) * 512],
            start=True, stop=True,
        )
        dst = ln_sb[:, d, c * 512:(c + 1) * 512]
        nc.vector.scalar_tensor_tensor(
            out=dst,
            in0=dst,
            scalar=g_sb[:, d:d + 1],
            in1=rb[:, :],
            op0=mybir.AluOpType.mult,
            op1=mybir.AluOpType.mult,
        )
        if b_sb is not None:
            nc.vector.tensor_scalar_add(dst, dst, b_sb[:, d:d + 1])

    # ---------------- P1: projections, finalize overlapped ----------------
    # Emission: K MMs -> V MMs (K LN-finalize pieces interleaved) ->
    # Q c0 MMs -> Q c1 MMs (Q c0 finalize interleaved) -> Q c1 finalize.
    # The finalize work runs on DVE/ScalarE under the next group's matmuls
    # so the PE never drains long enough for HAM to re-throttle.
    p1 = ExitStack()
    ps_proj = p1.enter_context(tc.tile_pool(name="ps_proj", bufs=4,
                                            space="PSUM"))
    ps_ssq = p1.enter_context(tc.tile_pool(name="ps_ssq", bufs=2,
                                           space="PSUM"))
    ps_rb = p1.enter_context(tc.tile_pool(name="ps_rb", bufs=2, space="PSUM"))

    ssq_k = proj_mms(ps_proj, ps_ssq, wk, enct, ktln, [0, 1], stage_act=True)

    k_fin = []
    for c in range(2):
        for d in range(8):
            k_fin.append((c, d))

    proj_rstd(ssq_k[0], rinv_k, 0)
    proj_rstd(ssq_k[1], rinv_k, 1)

    # V projection into augmented layout [kv, h, V|1], K finalize interleaved
    for tt in range(8):
        accs = [ps_proj.tile([128, 512], F32, tag="acc", name=f"acc{i}")
                for i in range(2)]
        for k in range(8):
            for c in range(2):
                nc.tensor.matmul(
                    accs[c][:, :],
                    lhsT=enct[:, k, tt * 128:(tt + 1) * 128],
                    rhs=wv[:, k, c * 512:(c + 1) * 512],
                    start=(k == 0), stop=(k == 7),
                )
        for c in range(2):
            dst = vaug[:, tt, 8 * c:8 * (c + 1), 0:HD]
            nc.scalar.copy(
                dst, accs[c][:, :].rearrange("p (h e) -> p h e", e=HD))
        if tt >= 3 and tt < 7:
            for c, d in k_fin[(tt - 3) * 4:(tt - 2) * 4]:
                proj_fin_d(ps_rb, "rb", ktln, gk_sb, bk_sb, rinv_k, c, d)

    # Q chunk-0 projection
    ssq_q0 = proj_mms(ps_proj, ps_ssq, wq, hst, qtln, [0], stage_act=True)
    proj_rstd(ssq_q0[0], rinv_q, 0)

    # Q chunk-1 projection with chunk-0 finalize interleaved
    ssqs_q1 = {}
    ssqs_q1[1] = ps_ssq.tile([16, 512], F32, tag="ssq", name="ssq1")
    for d in range(8):
        acc = ps_proj.tile([128, 512], F32, tag="acc", name="acc1")
        for k in range(8):
            nc.tensor.matmul(
                acc[:, :],
                lhsT=wq[:, k, d * 128:(d + 1) * 128],
                rhs=hst[:, k, 512:1024],
                start=(k == 0), stop=(k == 7),
            )
        nc.scalar.copy(qtln[:, d, 512:1024], acc[:, :])
        sq = sq_pool.tile([128, 512], F32R)
        nc.vector.tensor_mul(sq[:, :], qtln[:, d, 512:1024],
                             qtln[:, d, 512:1024])
        nc.tensor.matmul(
            ssqs_q1[1][:, :],
            lhsT=sel16_sb[:, d, :],
            rhs=sq[:, :],
            start=(d == 0), stop=(d == 7),
            skip_group_check=True,
        )
        if d >= 2:
            proj_fin_d(ps_rb, "rb", qtln, gq_sb, bq_sb, rinv_q, 0, d - 2)
    rstd_q1 = proj_rstd(ssqs_q1[1], rinv_q, 1)
    warm = rstd_pool.tile([16, 1], F32, tag="warm")
    nc.scalar.activation(warm[:, :], rstd_q1[:, 0:1],
                         mybir.ActivationFunctionType.Exp)
    for d in range(6, 8):
        proj_fin_d(ps_rb, "rb", qtln, gq_sb, bq_sb, rinv_q, 0, d)

    p1.close()

    # ---------------- P2: attention ----------------
    at_pool = ctx.enter_context(tc.tile_pool(name="at_pool", bufs=4))
    srow_pool = ctx.enter_context(tc.tile_pool(name="srow_pool", bufs=2))
    p2 = ExitStack()
    ps_sc = p2.enter_context(tc.tile_pool(name="ps_sc", bufs=2, space="PSUM"))
    ps_av = p2.enter_context(tc.tile_pool(name="ps_av", bufs=2, space="PSUM"))
    ps_fin = p2.enter_context(tc.tile_pool(name="ps_fin", bufs=2,
                                           space="PSUM"))

    def emit_av(p, quarter, ats, avs):
        for vv in range(2):
            v = 2 * quarter + vv
            for j in range(2):
                nc.tensor.matmul(
                    avs[j][:, :],
                    lhsT=vaug[:, v, 2 * p + j, :],
                    rhs=ats[j][:, vv, :],
                    start=(v == 0), stop=(v == 7),
                    skip_group_check=True,
                )

    out_pool = ctx.enter_context(tc.tile_pool(name="out_pool", bufs=2))

    def p3_tt(tt):
        # output projection for token tile tt (needs all of aout scaled)
        for cc in range(2):
            oacc = ps_fin.tile([128, 512], F32, tag="fin")
            for k in range(8):
                nc.tensor.matmul(
                    oacc[:, :],
                    lhsT=aout[:, k, tt * 128:(tt + 1) * 128],
                    rhs=wo[:, k, cc * 512:(cc + 1) * 512],
                    start=(k == 0), stop=(k == 7),
                )
            ot = out_pool.tile([128, 512], F32)
            nc.vector.tensor_copy(ot[:, :], oacc[:, :])
            nc.sync.dma_start(
                t["out"][tt * 128:(tt + 1) * 128,
                         cc * 512:(cc + 1) * 512],
                ot[:, :],
            )

    def p25_piece(c, p):
        # refresh 1/sums for this chunk (rows of still-undrained heads are
        # junk, but selB picks only rows 2p/2p+1) and scale aout block p
        recip_fast(inv_s[:, c * CH:(c + 1) * CH],
                   sums_sb[:, c * CH:(c + 1) * CH])
        rb = ps_fin.tile([128, CH], F32, tag="fin")
        nc.tensor.matmul(
            rb[:, :],
            lhsT=selB_sb[:, p, :],
            rhs=inv_s[:, c * CH:(c + 1) * CH],
            start=True, stop=True,
        )
        sl = aout[:, p, c * CH:(c + 1) * CH]
        nc.vector.tensor_mul(sl, sl, rb[:, :])

    for c in range(M // CH):
        for p in range(8):
            if p > 0:
                p25_piece(c, p - 1)
                if c == 1 and p % 2 == 1:
                    p3_tt((p - 1) // 2)
            elif c == 1:
                p25_piece(0, 7)
            avs = {j: ps_av.tile([HD + 1, CH], F32, tag="av", name=f"av{j}")
                   for j in range(2)}
            pend = None
            for quarter in range(4):
                scs = {j: ps_sc.tile([128, 2, CH], F32, tag="sc",
                                     name=f"sc{j}") for j in range(2)}
                for vv in range(2):
                    v = 2 * quarter + vv
                    for j in range(2):
                        # K=64: row-tile the two heads onto disjoint PE
                        # row-groups so they run concurrently
                        nc.tensor.matmul(
                            scs[j][:, vv, :],
                            lhsT=ktln[j * 64:(j + 1) * 64, p,
                                      v * 128:(v + 1) * 128],
                            rhs=qtln[j * 64:(j + 1) * 64, p,
                                     c * CH:(c + 1) * CH],
                            start=True, stop=True,
                            tile_position=(j * 64, 0),
                        )
                if c == 0 and p == 0 and quarter == 0:
                    # Q chunk-1 LN apply: keeps the PE busy across the
                    # exp-table-load stall at attention start
                    for d in range(8):
                        proj_fin_d(ps_fin, "fin", qtln, gq_sb, bq_sb,
                                   rinv_q, 1, d)
                # AV of the previous quarter AFTER this quarter's scores:
                # in the in-order PE queue the scores never sit behind an
                # AV that is still waiting on its exp
                if pend is not None:
                    emit_av(p, pend[0], pend[1], avs)
                ats = {}
                for j in range(2):
                    at = at_pool.tile([128, 2, CH], BF)
                    nc.scalar.activation(
                        at[:, :, :], scs[j][:, :, :],
                        mybir.ActivationFunctionType.Exp, scale=0.125,
                    )
                    ats[j] = at
                pend = (quarter, ats)
            emit_av(p, pend[0], pend[1], avs)
            # drain: attn out + softmax sums (row HD of augmented AV).
            # Engines need 32-aligned partition bases, so stage the sum
            # row at partition 0 and DMA-scatter into head-row h.
            for j in range(2):
                h = 2 * p + j
                av = avs[j]
                nc.vector.tensor_copy(
                    aout[j * 64:(j + 1) * 64, p, c * CH:(c + 1) * CH],
                    av[0:HD, :])
                srow = srow_pool.tile([1, CH], F32)
                nc.vector.tensor_copy(srow[:, :], av[HD:HD + 1, :])
                nc.sync.dma_start(
                    sums_sb[h:h + 1, c * CH:(c + 1) * CH], srow[:, :])
    p25_piece(1, 7)
    for tt in range(4, 8):
        p3_tt(tt)
    p2.close()


def _build(has_bias_q, has_bias_k):
    key = (has_bias_q, has_bias_k)
    if key in _cache:
        return _cache[key]
    nc = bacc.Bacc("TRN2", target_bir_lowering=False, debug=False,
                   num_devices=NCORES)
    t = {}

    def inp(name, shape, dt):
        t[name] = nc.dram_tensor(name, list(shape), dt, kind="ExternalInput").ap()

    inp("hsT", (D, M), BF)
    inp("encT", (D, SKV), BF)
    inp("wq", (D, D), BF)
    inp("wk", (D, D), BF)
    inp("wv", (D, D), BF)
    inp("wo", (D, D), BF)
    inp("gq", (D,), F32)
    inp("gk", (D,), F32)
    if has_bias_q:
        inp("bq", (D,), F32)
    if has_bias_k:
        inp("bk", (D,), F32)
    inp("sel16", (8, 128, H), F32R)
    inp("selB", (8, H, 128), F32R)
    t["out"] = nc.dram_tensor("out", [M, D], F32, kind="ExternalOutput").ap()

    with tile.TileContext(nc) as tc:
        with ExitStack() as ctx:
            _emit(ctx, tc, t, has_bias_q, has_bias_k)
    nc.finalize()
    _cache[key] = nc
    return nc


def _center_fold(W):
    # Fold per-head output-column mean removal into the weight matrix (exact).
    Wr = np.asarray(W, np.float32).reshape(D, H, HD)
    return (Wr - Wr.mean(axis=2, keepdims=True)).reshape(D, D)


def kernel(hidden_states, encoder_hidden_states, Wq, Wk, Wv, Wo,
           gq, bq, gk, bk, _trace=False):
    hs = np.asarray(hidden_states, np.float32)
    enc = np.asarray(encoder_hidden_states, np.float32)
    bq = np.asarray(bq, np.float32)
    bk = np.asarray(bk, np.float32)
    has_bias_q = bool(np.any(bq != 0))
    has_bias_k = bool(np.any(bk != 0))
    nc = _build(has_bias_q, has_bias_k)

    bf = ml_dtypes.bfloat16
    wq_bf = _center_fold(Wq).astype(bf)
    wk_bf = _center_fold(Wk).astype(bf)
    wv_bf = np.asarray(Wv, np.float32).astype(bf)
    wo_bf = np.asarray(Wo, np.float32).astype(bf)
    gq_rep = np.tile(np.asarray(gq, np.float32), H)
    gk_rep = np.tile(np.asarray(gk, np.float32), H)
    sel16, selB = _selector_constants()

    common = {
        "wq": wq_bf, "wk": wk_bf, "wv": wv_bf, "wo": wo_bf,
        "gq": gq_rep, "gk": gk_rep,
        "sel16": sel16, "selB": selB,
    }
    if has_bias_q:
        common["bq"] = np.tile(bq, H)
    if has_bias_k:
        common["bk"] = np.tile(bk, H)

    in_maps = []
    for core in range(NCORES):
        b, qb = divmod(core, 4)
        hsT = np.ascontiguousarray(
            hs[b, qb * M:(qb + 1) * M, :].T).astype(bf)
        encT = np.ascontiguousarray(enc[b].T).astype(bf)
        in_maps.append({**common, "hsT": hsT, "encT": encT})

    res = run_bass_kernel_spmd(nc, in_maps, list(range(NCORES)), trace=_trace)

    out = np.empty((B, SQ, D), np.float32)
    for core in range(NCORES):
        b, qb = divmod(core, 4)
        out[b, qb * M:(qb + 1) * M, :] = res.results[core]["out"]
    kernel.last_exec_time_ns = res.exec_time_ns
    kernel.last_results = res
    return out


# revision 22
# speedup vs baseline: 1.1210x; 1.1210x over previous
"""CrossAttention Trainium2 kernel (8 NeuronCores, SPMD).

Sharding: 8 cores = batch(2) x query-block(4 x 1024). Each core computes a
[1024, 1024] slice of the output; no cross-core communication.

Reference math (per core, M=1024 query tokens, Skv=1024, D=1024, H=16, hd=64):
  q = hs @ Wq ; k = enc @ Wk ; v = enc @ Wv
  per-head LN(q), LN(k) over hd; scores = LN(q) @ LN(k)^T / sqrt(hd)
  out = softmax(scores) @ v ; return out @ Wo
Host folds the LN mean-centering into Wq/Wk (exact), pre-transposes
activations to feature-major, and casts matmul operands to bf16.

Schedule: K-proj MMs -> V-proj MMs (K LN-finalize overlaps on DVE) ->
Q-chunk0 proj -> attention c0 (Q-chunk1 proj d-groups sprinkled into the
p-loop; exp on ScalarE is the rate limiter) -> boundary (Q-c1 LN finalize,
inv-softmax c0) -> attention c1 (remaining Q-c1 finalize, P2.5 scale c0 and
O-proj c0 sprinkled in) -> tail (scale c1, O-proj c1). Score matmuls are
K=64 row-tiled so both heads of a p-group run concurrently on the PE.
"""

import numpy as np
import ml_dtypes
from contextlib import ExitStack

import concourse.bass as bass
import concourse.tile as tile
from concourse import bacc, mybir
from concourse.bass_utils import run_bass_kernel_spmd

BF = mybir.dt.bfloat16
F32 = mybir.dt.float32
F32R = mybir.dt.float32r

D = 1024      # model dim
H = 16        # heads
HD = 64       # head dim
M = 1024      # query tokens per core
SKV = 1024    # kv tokens (one batch)
B = 2
SQ = 4096
NCORES = 8
LN_EPS = 1e-5
CH = 512      # query-token chunk

_cache = {}


def _selector_constants():
    # sel16[d][p, j]: 1 if head j == 2d + p//64  (sum-over-head-partitions lhsT)
    sel16 = np.zeros((8, 128, H), np.float32)
    for d in range(8):
        for p in range(128):
            sel16[d, p, 2 * d + p // 64] = 1.0
    # selB[d][j, p]: 1 if head j == 2d + p//64  (broadcast-to-head-partitions lhsT)
    selB = np.transpose(sel16, (0, 2, 1)).copy()
    return sel16, selB


def _emit(ctx: ExitStack, tc, t, has_bias_q, has_bias_k):
    nc = tc.nc

    persist = ctx.enter_context(tc.tile_pool(name="persist", bufs=1))

    # ---- persistent SBUF tensors (stacked [128, 8, 1024] layout) ----
    hst = persist.tile([128, 8, M], BF, tag="hst")       # hs^T  (feature-major)
    enct = persist.tile([128, 8, SKV], BF, tag="enct")   # enc^T (feature-major)
    wq = persist.tile([128, 8, D], BF, tag="wq")
    wk = persist.tile([128, 8, D], BF, tag="wk")
    wv = persist.tile([128, 8, D], BF, tag="wv")
    wo = persist.tile([128, 8, D], BF, tag="wo")
    qtln = persist.tile([128, 8, M], BF, tag="qtln")     # LN(q)^T feature-major
    ktln = persist.tile([128, 8, SKV], BF, tag="ktln")   # LN(k)^T feature-major
    vaug = persist.tile([128, 8, H, HD + 1], BF, tag="vaug")  # [kv, h, V|1]
    aout = persist.tile([128, 8, M], BF, tag="aout")     # attn out^T feature-major
    gq_sb = persist.tile([128, 8], F32, tag="gq_sb")
    gk_sb = persist.tile([128, 8], F32, tag="gk_sb")
    sel16_sb = persist.tile([128, 8, H], F32R, tag="sel16_sb")
    selB_sb = persist.tile([16, 8, 128], F32R, tag="selB_sb")
    rinv_q = persist.tile([16, M], F32R, tag="rinv_q")    # 1/std per (head, tok)
    rinv_k = persist.tile([16, SKV], F32R, tag="rinv_k")
    sums_sb = persist.tile([16, M], F32, tag="sums_sb")   # softmax sums
    inv_s = persist.tile([16, M], F32R, tag="inv_s")      # 1/softmax-sum
    eps_sb = persist.tile([16, 1], F32, tag="eps_sb")
    nc.vector.memset(eps_sb[:, :], LN_EPS)
    nc.vector.memset(sums_sb[:, :], 1.0)  # recip_fast is undefined on junk
    nc.vector.memset(vaug[:, :, :, HD:HD + 1], 1.0)
    bq_sb = persist.tile([128, 8], F32, tag="bq_sb") if has_bias_q else None
    bk_sb = persist.tile([128, 8], F32, tag="bk_sb") if has_bias_k else None

    # ---- loads ----
    for k in range(8):
        nc.sync.dma_start(enct[:, k, :], t["encT"][k * 128:(k + 1) * 128, :])
        nc.sync.dma_start(wk[:, k, :], t["wk"][k * 128:(k + 1) * 128, :])
    for k in range(8):
        nc.sync.dma_start(wv[:, k, :], t["wv"][k * 128:(k + 1) * 128, :])
        nc.sync.dma_start(hst[:, k, :], t["hsT"][k * 128:(k + 1) * 128, :])
    for k in range(8):
        nc.sync.dma_start(wq[:, k, :], t["wq"][k * 128:(k + 1) * 128, :])
        nc.sync.dma_start(wo[:, k, :], t["wo"][k * 128:(k + 1) * 128, :])
    nc.sync.dma_start(gq_sb[:, :], t["gq"].rearrange("(d p) -> p d", p=128))
    nc.sync.dma_start(gk_sb[:, :], t["gk"].rearrange("(d p) -> p d", p=128))
    if has_bias_q:
        nc.sync.dma_start(bq_sb[:, :], t["bq"].rearrange("(d p) -> p d", p=128))
    if has_bias_k:
        nc.sync.dma_start(bk_sb[:, :], t["bk"].rearrange("(d p) -> p d", p=128))
    nc.sync.dma_start(sel16_sb[:, :, :], t["sel16"].rearrange("d p j -> p d j"))
    nc.sync.dma_start(selB_sb[:, :, :], t["selB"].rearrange("d j p -> j d p"))

    def recip_fast(out_ap, in_ap):
        from concourse.dve_ops import (
            RECIP_APPROX_FAST_CONSTS,
            RECIPROCAL_APPROX_FAST,
        )
        c = RECIP_APPROX_FAST_CONSTS
        nc.vector._custom_dve(
            RECIPROCAL_APPROX_FAST, out=out_ap, in0=in_ap,
            s0=c["s0"], s1=c["s1"], imm2=c["imm2"],
        )

    sq_pool = ctx.enter_context(tc.tile_pool(name="sq_pool", bufs=2))
    rstd_pool = ctx.enter_context(tc.tile_pool(name="rstd_pool", bufs=2))

    # ---------------- projection helpers ----------------
    def proj_mms(ps_proj, ps_ssq, w_sb, x_sb, ln_sb, cg, stage_act):
        """Emit projection matmuls + staging + per-head sum-of-squares for
        token chunks cg. Returns the ssq PSUM tiles (per chunk)."""
        ssqs = {}
        for c in cg:
            ssqs[c] = ps_ssq.tile([16, 512], F32, tag="ssq", name=f"ssq{c}")
        for d in range(8):
            accs = {c: ps_proj.tile([128, 512], F32, tag="acc",
                                    name=f"acc{c}") for c in cg}
            for k in range(8):
                for c in cg:
                    nc.tensor.matmul(
                        accs[c][:, :],
                        lhsT=w_sb[:, k, d * 128:(d + 1) * 128],
                        rhs=x_sb[:, k, c * 512:(c + 1) * 512],
                        start=(k == 0), stop=(k == 7),
                    )
            for c in cg:
                acc = accs[c]
                # stage raw projection (bf16); LN apply rescales in place
                if stage_act:
                    nc.scalar.copy(ln_sb[:, d, c * 512:(c + 1) * 512],
                                   acc[:, :])
                else:
                    nc.vector.tensor_copy(
                        ln_sb[:, d, c * 512:(c + 1) * 512], acc[:, :])
                sq = sq_pool.tile([128, 512], F32R)
                nc.vector.tensor_mul(sq[:, :],
                                     ln_sb[:, d, c * 512:(c + 1) * 512],
                                     ln_sb[:, d, c * 512:(c + 1) * 512])
                nc.tensor.matmul(
                    ssqs[c][:, :],
                    lhsT=sel16_sb[:, d, :],
                    rhs=sq[:, :],
                    start=(d == 0), stop=(d == 7),
                    skip_group_check=True,
                )
        return ssqs

    def proj_rstd(ssq, rinv_sb, c):
        # rstd = sqrt(ssq/64 + eps); rinv = 1/rstd
        rstd = rstd_pool.tile([16, 512], F32)
        nc.scalar.activation(
            rstd[:, :], ssq[:, :], mybir.ActivationFunctionType.Sqrt,
            bias=eps_sb[:, :], scale=1.0 / HD,
        )
        recip_fast(rinv_sb[:, c * 512:(c + 1) * 512], rstd[:, :])
        return rstd

    def proj_fin_d(ps_rb, rb_tag, ln_sb, g_sb, b_sb, rinv_sb, c, d):
        # apply: ln = raw * g * rinv (+ b) for feature block d of chunk c
        rb = ps_rb.tile([128, 512], F32, tag=rb_tag)
        nc.tensor.matmul(
            rb[:, :],
            lhsT=selB_sb[:, d, :],
            rhs=rinv_sb[:, c * 512:(c + 1) * 512],
            start=True, stop=True,
        )
        dst = ln_sb[:, d, c * 512:(c + 1) * 512]
        nc.vector.scalar_tensor_tensor(
            out=dst,
            in0=dst,
            scalar=g_sb[:, d:d + 1],
            in1=rb[:, :],
            op0=mybir.AluOpType.mult,
            op1=mybir.AluOpType.mult,
        )
        if b_sb is not None:
            nc.vector.tensor_scalar_add(dst, dst, b_sb[:, d:d + 1])

    # ---------------- P1: projections, finalize overlapped ----------------
    # Emission: K MMs -> V MMs (K LN-finalize pieces interleaved) ->
    # Q c0 MMs -> Q c1 MMs (Q c0 finalize interleaved) -> Q c1 finalize.
    # The finalize work runs on DVE/ScalarE under the next group's matmuls
    # so the PE never drains long enough for HAM to re-throttle.
    p1 = ExitStack()
    ps_proj = p1.enter_context(tc.tile_pool(name="ps_proj", bufs=4,
                                            space="PSUM"))
    ps_ssq = p1.enter_context(tc.tile_pool(name="ps_ssq", bufs=2,
                                           space="PSUM"))
    ps_rb = p1.enter_context(tc.tile_pool(name="ps_rb", bufs=2, space="PSUM"))

    ssq_k = proj_mms(ps_proj, ps_ssq, wk, enct, ktln, [0, 1], stage_act=True)

    k_fin = []
    for c in range(2):
        for d in range(8):
            k_fin.append((c, d))

    proj_rstd(ssq_k[0], rinv_k, 0)
    proj_rstd(ssq_k[1], rinv_k, 1)

    # V projection into augmented layout [kv, h, V|1], K finalize interleaved
    for tt in range(8):
        accs = [ps_proj.tile([128, 512], F32, tag="acc", name=f"acc{i}")
                for i in range(2)]
        for k in range(8):
            for c in range(2):
                nc.tensor.matmul(
                    accs[c][:, :],
                    lhsT=enct[:, k, tt * 128:(tt + 1) * 128],
                    rhs=wv[:, k, c * 512:(c + 1) * 512],
                    start=(k == 0), stop=(k == 7),
                )
        for c in range(2):
            dst = vaug[:, tt, 8 * c:8 * (c + 1), 0:HD]
            nc.scalar.copy(
                dst, accs[c][:, :].rearrange("p (h e) -> p h e", e=HD))
        if tt >= 3 and tt < 7:
            for c, d in k_fin[(tt - 3) * 4:(tt - 2) * 4]:
                proj_fin_d(ps_rb, "rb", ktln, gk_sb, bk_sb, rinv_k, c, d)

    # Q chunk-0 projection
    ssq_q0 = proj_mms(ps_proj, ps_ssq, wq, hst, qtln, [0], stage_act=True)
    proj_rstd(ssq_q0[0], rinv_q, 0)

    # Q chunk-1 projection with chunk-0 finalize interleaved
    ssqs_q1 = {}
    ssqs_q1[1] = ps_ssq.tile([16, 512], F32, tag="ssq", name="ssq1")
    for d in range(8):
        acc = ps_proj.tile([128, 512], F32, tag="acc", name="acc1")
        for k in range(8):
            nc.tensor.matmul(
                acc[:, :],
                lhsT=wq[:, k, d * 128:(d + 1) * 128],
                rhs=hst[:, k, 512:1024],
                start=(k == 0), stop=(k == 7),
            )
        nc.scalar.copy(qtln[:, d, 512:1024], acc[:, :])
        sq = sq_pool.tile([128, 512], F32R)
        nc.vector.tensor_mul(sq[:, :], qtln[:, d, 512:1024],
                             qtln[:, d, 512:1024])
        nc.tensor.matmul(
            ssqs_q1[1][:, :],
            lhsT=sel16_sb[:, d, :],
            rhs=sq[:, :],
            start=(d == 0), stop=(d == 7),
            skip_group_check=True,
        )
        if d >= 2:
            proj_fin_d(ps_rb, "rb", qtln, gq_sb, bq_sb, rinv_q, 0, d - 2)
    rstd_q1 = proj_rstd(ssqs_q1[1], rinv_q, 1)
    warm = rstd_pool.tile([16, 1], F32, tag="warm")
    nc.scalar.activation(warm[:, :], rstd_q1[:, 0:1],
                         mybir.ActivationFunctionType.Exp)
    for d in range(6, 8):
        proj_fin_d(ps_rb, "rb", qtln, gq_sb, bq_sb, rinv_q, 0, d)

    p1.close()

    # ---------------- P2: attention ----------------
    at_pool = ctx.enter_context(tc.tile_pool(name="at_pool", bufs=4))
    srow_pool = ctx.enter_context(tc.tile_pool(name="srow_pool", bufs=2))
    p2 = ExitStack()
    ps_sc = p2.enter_context(tc.tile_pool(name="ps_sc", bufs=2, space="PSUM"))
    ps_av = p2.enter_context(tc.tile_pool(name="ps_av", bufs=2, space="PSUM"))
    ps_fin = p2.enter_context(tc.tile_pool(name="ps_fin", bufs=2,
                                           space="PSUM"))

    def emit_av(p, quarter, ats, avs):
        for vv in range(2):
            v = 2 * quarter + vv
            for j in range(2):
                nc.tensor.matmul(
                    avs[j][:, :],
                    lhsT=vaug[:, v, 2 * p + j, :],
                    rhs=ats[j][:, vv, :],
                    start=(v == 0), stop=(v == 7),
                    skip_group_check=True,
                )

    def p25_piece(c, p):
        # refresh 1/sums for this chunk (rows of still-undrained heads are
        # junk, but selB picks only rows 2p/2p+1) and scale aout block p
        recip_fast(inv_s[:, c * CH:(c + 1) * CH],
                   sums_sb[:, c * CH:(c + 1) * CH])
        rb = ps_fin.tile([128, CH], F32, tag="fin")
        nc.tensor.matmul(
            rb[:, :],
            lhsT=selB_sb[:, p, :],
            rhs=inv_s[:, c * CH:(c + 1) * CH],
            start=True, stop=True,
        )
        sl = aout[:, p, c * CH:(c + 1) * CH]
        nc.vector.tensor_mul(sl, sl, rb[:, :])

    for c in range(M // CH):
        for p in range(8):
            if p > 0:
                p25_piece(c, p - 1)
            elif c == 1:
                p25_piece(0, 7)
            avs = {j: ps_av.tile([HD + 1, CH], F32, tag="av", name=f"av{j}")
                   for j in range(2)}
            pend = None
            for quarter in range(4):
                scs = {j: ps_sc.tile([128, 2, CH], F32, tag="sc",
                                     name=f"sc{j}") for j in range(2)}
                for vv in range(2):
                    v = 2 * quarter + vv
                    for j in range(2):
                        # K=64: row-tile the two heads onto disjoint PE
                        # row-groups so they run concurrently
                        nc.tensor.matmul(
                            scs[j][:, vv, :],
                            lhsT=ktln[j * 64:(j + 1) * 64, p,
                                      v * 128:(v + 1) * 128],
                            rhs=qtln[j * 64:(j + 1) * 64, p,
                                     c * CH:(c + 1) * CH],
                            start=True, stop=True,
                            tile_position=(j * 64, 0),
                        )
                if c == 0 and p == 0 and quarter == 0:
                    # Q chunk-1 LN apply: keeps the PE busy across the
                    # exp-table-load stall at attention start
                    for d in range(8):
                        proj_fin_d(ps_fin, "fin", qtln, gq_sb, bq_sb,
                                   rinv_q, 1, d)
                # AV of the previous quarter AFTER this quarter's scores:
                # in the in-order PE queue the scores never sit behind an
                # AV that is still waiting on its exp
                if pend is not None:
                    emit_av(p, pend[0], pend[1], avs)
                ats = {}
                for j in range(2):
                    at = at_pool.tile([128, 2, CH], BF)
                    nc.scalar.activation(
                        at[:, :, :], scs[j][:, :, :],
                        mybir.ActivationFunctionType.Exp, scale=0.125,
                    )
                    ats[j] = at
                pend = (quarter, ats)
            emit_av(p, pend[0], pend[1], avs)
            # drain: attn out + softmax sums (row HD of augmented AV).
            # Engines need 32-aligned partition bases, so stage the sum
            # row at partition 0 and DMA-scatter into head-row h.
            for j in range(2):
                h = 2 * p + j
                av = avs[j]
                nc.vector.tensor_copy(
                    aout[j * 64:(j + 1) * 64, p, c * CH:(c + 1) * CH],
                    av[0:HD, :])
                srow = srow_pool.tile([1, CH], F32)
                nc.vector.tensor_copy(srow[:, :], av[HD:HD + 1, :])
                nc.sync.dma_start(
                    sums_sb[h:h + 1, c * CH:(c + 1) * CH], srow[:, :])
    p25_piece(1, 7)
    p2.close()

    # ---------------- P3: output projection ----------------
    out_pool = ctx.enter_context(tc.tile_pool(name="out_pool", bufs=2))
    with tc.tile_pool(name="ps_out", bufs=4, space="PSUM") as ps_out:
        for c in range(M // CH):
            for tt in range(4 * c, 4 * (c + 1)):
                accs = {cc: ps_out.tile([128, 512], F32, tag="oacc",
                                        name=f"oacc{cc}") for cc in range(2)}
                for k in range(8):
                    for cc in range(2):
                        nc.tensor.matmul(
                            accs[cc][:, :],
                            lhsT=aout[:, k, tt * 128:(tt + 1) * 128],
                            rhs=wo[:, k, cc * 512:(cc + 1) * 512],
                            start=(k == 0), stop=(k == 7),
                        )
                for cc in range(2):
                    ot = out_pool.tile([128, 512], F32)
                    nc.vector.tensor_copy(ot[:, :], accs[cc][:, :])
                    nc.sync.dma_start(
                        t["out"][tt * 128:(tt + 1) * 128,
                                 cc * 512:(cc + 1) * 512],
                        ot[:, :],
                    )


def _build(has_bias_q, has_bias_k):
    key = (has_bias_q, has_bias_k)
    if key in _cache:
        return _cache[key]
    nc = bacc.Bacc("TRN2", target_bir_lowering=False, debug=False,
                   num_devices=NCORES)
    t = {}

    def inp(name, shape, dt):
        t[name] = nc.dram_tensor(name, list(shape), dt, kind="ExternalInput").ap()

    inp("hsT", (D, M), BF)
    inp("encT", (D, SKV), BF)
    inp("wq", (D, D), BF)
    inp("wk", (D, D), BF)
    inp("wv", (D, D), BF)
    inp("wo", (D, D), BF)
    inp("gq", (D,), F32)
    inp("gk", (D,), F32)
    if has_bias_q:
        inp("bq", (D,), F32)
    if has_bias_k:
        inp("bk", (D,), F32)
    inp("sel16", (8, 128, H), F32R)
    inp("selB", (8, H, 128), F32R)
    t["out"] = nc.dram_tensor("out", [M, D], F32, kind="ExternalOutput").ap()

    with tile.TileContext(nc) as tc:
        with ExitStack() as ctx:
            _emit(ctx, tc, t, has_bias_q, has_bias_k)
    nc.finalize()
    _cache[key] = nc
    return nc


def _center_fold(W):
    # Fold per-head output-column mean removal into the weight matrix (exact).
    Wr = np.asarray(W, np.float32).reshape(D, H, HD)
    return (Wr - Wr.mean(axis=2, keepdims=True)).reshape(D, D)


def kernel(hidden_states, encoder_hidden_states, Wq, Wk, Wv, Wo,
           gq, bq, gk, bk, _trace=False):
    hs = np.asarray(hidden_states, np.float32)
    enc = np.asarray(encoder_hidden_states, np.float32)
    bq = np.asarray(bq, np.float32)
    bk = np.asarray(bk, np.float32)
    has_bias_q = bool(np.any(bq != 0))
    has_bias_k = bool(np.any(bk != 0))
    nc = _build(has_bias_q, has_bias_k)

    bf = ml_dtypes.bfloat16
    wq_bf = _center_fold(Wq).astype(bf)
    wk_bf = _center_fold(Wk).astype(bf)
    wv_bf = np.asarray(Wv, np.float32).astype(bf)
    wo_bf = np.asarray(Wo, np.float32).astype(bf)
    gq_rep = np.tile(np.asarray(gq, np.float32), H)
    gk_rep = np.tile(np.asarray(gk, np.float32), H)
    sel16, selB = _selector_constants()

    common = {
        "wq": wq_bf, "wk": wk_bf, "wv": wv_bf, "wo": wo_bf,
        "gq": gq_rep, "gk": gk_rep,
        "sel16": sel16, "selB": selB,
    }
    if has_bias_q:
        common["bq"] = np.tile(bq, H)
    if has_bias_k:
        common["bk"] = np.tile(bk, H)

    in_maps = []
    for core in range(NCORES):
        b, qb = divmod(core, 4)
        hsT = np.ascontiguousarray(
            hs[b, qb * M:(qb + 1) * M, :].T).astype(bf)
        encT = np.ascontiguousarray(enc[b].T).astype(bf)
        in_maps.append({**common, "hsT": hsT, "encT": encT})

    res = run_bass_kernel_spmd(nc, in_maps, list(range(NCORES)), trace=_trace)

    out = np.empty((B, SQ, D), np.float32)
    for core in range(NCORES):
        b, qb = divmod(core, 4)
        out[b, qb * M:(qb + 1) * M, :] = res.results[core]["out"]
    kernel.last_exec_time_ns = res.exec_time_ns
    kernel.last_results = res
    return out
